# revision 17
# baseline (speedup 1.0000x reference)
"""Causal single-head attention (B=4, T=4096, D_in=1024, D_out=64) on 8 trn2 cores.

Sharding: 2 cores per batch. Within a pair, core h in {0,1} owns the k/v
positions in 256-wide blocks of parity h (even/odd), and computes partial
unnormalized attention for ALL 4096 queries over its k half, plus the
softmax row-sums (via a ones-column appended to V). The host sums the two
partials and normalizes. Causality lands symmetrically on both parities, so
one SPMD program serves all 8 cores; per-core behavior differs only through
data (the parity block swap baked into the host-side xT layout, and maskD,
the precomputed diagonal-pair causal mask).

Scheduling (v6):
  - ALL HBM loads ride the sync HWDGE ring, triggered up front in need
    order: wkv, stripe0-half-a, wq2, stripe0-half-b, stripes 1-3. The ring
    is FIFO and no trigger waits on compute, so it streams continuously;
    per-partition runs are 8KB (stripe-0 halves) / 16KB (stripes 1-3)
    because the SDMA engines are descriptor-latency bound. The out stores
    trail the loads on the same ring.
  - The scalar (ACT) ring carries only the small V xbar-transposes (the
    last one rides sync); gpsimd carries constants + kT dups + the
    diagonal-mask multiplies. ACTIVATE (exp) is never queued behind a
    big DMA trigger.
  - V is produced via DMA xbar-transpose ([64, N] fp16 -> contiguous
    scratch -> DVE copy) instead of PE transposes (~4.5us off the PE).
  - Attention pairs are emitted at pair granularity, interleaved into the
    projection chains (Q-half first) so each slot's scores start ~2us into
    its segment and ACT is fed during projections. attnV lags by 1 pair.
  - Junk fp16 matmuls at t=0 keep the PE HAM clock-gate open until the
    first stripe half lands.

Attention per q-slot qt (512 queries, 2*qt+2 k-tiles of 128): scores use
c=64 contraction, two k-tiles packed in the PE row halves (tile_position
(0,0)/(64,0)), K^T and Q^T duplicated into partitions 64..127 (K^T by DMA
dup, Q^T free via doubled Wq columns). The diagonal (masked) pair's second
k-tile is trimmed to the causally needed 384 q-columns. attnV is m=65 (V
plus a ones column for the softmax denominator).
"""

import sys
import types

import numpy as np

B, T, D, E = 4, 4096, 1024, 64
NCORES = 8
P = 128
HB = 256  # parity half-block width
NQT = 8  # q-slots of 512
DC = D // P  # 8 d-chunks
NJUNK = 13

_cache = {}


def _sl(start, size):
    return slice(start, start + size)


def _build_program():
    import concourse.mybir as mybir
    import concourse.tile as tile
    from concourse import bacc

    f32 = mybir.dt.float32
    fp16 = mybir.dt.float16
    Exp = mybir.ActivationFunctionType.Exp
    Alu = mybir.AluOpType

    nc = bacc.Bacc("TRN2", target_bir_lowering=False, debug=False, num_devices=NCORES)

    xs0a = nc.dram_tensor("xs0a", [P, 2, DC, HB], fp16, kind="ExternalInput")
    xs0b = nc.dram_tensor("xs0b", [P, 2, DC, HB], fp16, kind="ExternalInput")
    xs1 = nc.dram_tensor("xs1", [P, 4, DC, HB], fp16, kind="ExternalInput")
    xs2 = nc.dram_tensor("xs2", [P, 4, DC, HB], fp16, kind="ExternalInput")
    xs3 = nc.dram_tensor("xs3", [P, 4, DC, HB], fp16, kind="ExternalInput")
    wkv = nc.dram_tensor("wkv", [P, DC, 2 * E], fp16, kind="ExternalInput")
    wq2 = nc.dram_tensor("wq2", [P, DC, P], fp16, kind="ExternalInput")
    maskD = nc.dram_tensor("maskD", [P, 896], fp16, kind="ExternalInput")
    ident = nc.dram_tensor("ident", [P, 64], fp16, kind="ExternalInput")
    ones = nc.dram_tensor("ones", [P, 16], fp16, kind="ExternalInput")
    out = nc.dram_tensor("out", [E + 1, T], f32, kind="ExternalOutput")

    with tile.TileContext(nc) as tc:
        with (
            tc.tile_pool(name="const", bufs=1) as cpool,
            tc.tile_pool(name="persist", bufs=1) as ppool,
            tc.tile_pool(name="xt0", bufs=2) as xt0pool,
            tc.tile_pool(name="xt", bufs=3) as xtpool,
            tc.tile_pool(name="vs", bufs=2) as vspool,
            tc.tile_pool(name="kvps", bufs=2, space="PSUM") as kvps,
            tc.tile_pool(name="sps", bufs=2, space="PSUM") as sps,
            tc.tile_pool(name="ops", bufs=2, space="PSUM") as ops,
            tc.tile_pool(name="exp", bufs=8) as exppool,
        ):
            kT_sb = ppool.tile([P, T // 2], fp16, name="kT")  # rows 64+: dup
            qT_sb = ppool.tile([P, T], fp16, name="qT")  # rows 64+: dup
            vT_tmp = ppool.tile([P, T // 2], fp16, name="vTt")  # rows 64+ used
            V_sb = ppool.tile([P, 16, E + 1], fp16, name="V")
            out_sb = ppool.tile([E + 1, T], f32, name="outsb")

            # PE warm-up: junk matmuls on a memset tile issue immediately
            # and hold the HAM clock-gate open until the first stripe lands.
            junk_in = ppool.tile([P, 512], fp16, name="junkin")
            nc.vector.memset(junk_in[:], 0.0)
            warm = ops.tile([E + 1, 512], f32, tag="po")
            for _ in range(NJUNK):
                nc.tensor.matmul(
                    warm[0:64, :],
                    junk_in[:, 0:E],
                    junk_in[:],
                    start=True,
                    stop=True,
                )

            # --- DMA triggers: all loads up front on the sync ring.
            wkv_sb = cpool.tile([P, DC, 2 * E], fp16)
            nc.sync.dma_start(wkv_sb[:], wkv.ap())
            xs0a_t = xt0pool.tile([P, 2, DC, HB], fp16, name="xs0t")
            nc.sync.dma_start(xs0a_t[:], xs0a.ap())
            wq2_sb = cpool.tile([P, DC, P], fp16)
            nc.sync.dma_start(wq2_sb[:], wq2.ap())
            xs0b_t = xt0pool.tile([P, 2, DC, HB], fp16, name="xs0t")
            nc.sync.dma_start(xs0b_t[:], xs0b.ap())
            xs_t = [None] * 4
            for s, xsrc in ((1, xs1), (2, xs2), (3, xs3)):
                xs_t[s] = xtpool.tile([P, 4, DC, HB], fp16, name="xst")
                nc.sync.dma_start(xs_t[s][:], xsrc.ap())
            maskD_sb = cpool.tile([P, 896], fp16)
            nc.gpsimd.dma_start(maskD_sb[:], maskD.ap())
            nc.gpsimd.dma_start(V_sb[:, :, E], ones.ap())  # ones column
            ident_sb = cpool.tile([P, 64], fp16)
            nc.gpsimd.dma_start(ident_sb[:], ident.ap())

            def kv_block_q0(m, src):
                # stripe-0 KV block m from half tensor src (own parity qtr)
                kvh = kvps.tile([P, HB], f32, tag="proj")
                for dc in range(DC):
                    nc.tensor.matmul(
                        kvh[:],
                        wkv_sb[:, dc, :],
                        src[:, 0, dc, :],
                        start=(dc == 0),
                        stop=(dc == DC - 1),
                    )
                nc.vector.tensor_copy(kT_sb[0:E, _sl(HB * m, HB)], kvh[0:E, :])
                nc.vector.tensor_copy(vT_tmp[E:P, _sl(HB * m, HB)], kvh[E:P, :])
                # per-block K^T dup + V transpose so slot-0/1 work isn't
                # gated on the whole stripe (scalar HWDGE: empty ring, low
                # latency; gpsimd SWDGE was ~2-3us late under sync load)
                nc.scalar.dma_start(
                    kT_sb[E:P, _sl(HB * m, HB)], kT_sb[0:E, _sl(HB * m, HB)]
                )
                for j in (2 * m, 2 * m + 1):
                    vt = kvps.tile([P, E], fp16, tag="proj")
                    nc.tensor.transpose(
                        vt[:], vT_tmp[E:P, _sl(P * j, P)], ident_sb[E:P, :]
                    )
                    nc.vector.tensor_copy(V_sb[:, j, 0:E], vt[:])

            def q_half_q0(half, src):
                q = kvps.tile([P, 512], f32, tag="proj")
                for dc in range(DC):
                    nc.tensor.matmul(
                        q[:],
                        wq2_sb[:, dc, :],
                        src[:, :, dc, :],
                        start=(dc == 0),
                        stop=(dc == DC - 1),
                    )
                nc.vector.tensor_copy(qT_sb[:, _sl(512 * half, 512)], q[:])

            def kv_stripe(t2):
                # stripes 1..3: K^T|V^T one MM per d-chunk over both parity
                # blocks (strided rhs), then copies + dup
                xts = xs_t[t2]
                xts_v = xts[:, :, :, :].rearrange("p (h par) c o -> p h par c o", h=2)
                kv = kvps.tile([P, 512], f32, tag="proj")
                for dc in range(DC):
                    nc.tensor.matmul(
                        kv[:],
                        wkv_sb[:, dc, :],
                        xts_v[:, :, 0, dc, :],
                        start=(dc == 0),
                        stop=(dc == DC - 1),
                    )
                m = 2 * t2
                nc.vector.tensor_copy(kT_sb[0:E, _sl(HB * m, 512)], kv[0:E, :])
                nc.vector.tensor_copy(vT_tmp[E:P, _sl(HB * m, 512)], kv[E:P, :])
                nc.scalar.dma_start(
                    kT_sb[E:P, _sl(512 * t2, 512)], kT_sb[0:E, _sl(512 * t2, 512)]
                )
                issue_vt(t2)

            def q_half(t2, half):
                xts = xs_t[t2]
                q = kvps.tile([P, 512], f32, tag="proj")
                for dc in range(DC):
                    nc.tensor.matmul(
                        q[:],
                        wq2_sb[:, dc, :],
                        xts[:, _sl(2 * half, 2), dc, :],
                        start=(dc == 0),
                        stop=(dc == DC - 1),
                    )
                qt_i = 2 * t2 + half
                nc.vector.tensor_copy(qT_sb[:, _sl(512 * qt_i, 512)], q[:])

            def issue_vt(t2):
                # V^T -> V via PE transpose (DMA xbar transpose would force
                # Tile's transpose-vs-DMA deadlock guard, serializing the
                # load stream behind compute)
                for j in range(4 * t2, 4 * t2 + 4):
                    vt = kvps.tile([P, E], fp16, tag="proj")
                    nc.tensor.transpose(
                        vt[:], vT_tmp[E:P, _sl(P * j, P)], ident_sb[E:P, :]
                    )
                    nc.vector.tensor_copy(V_sb[:, j, 0:E], vt[:])

            pendings = []  # (qt, oi, j0, nkb, exp_tile, po, w2)

            def issue_attnv(pend):
                qt, oi, j0, nkb, ex, po_t, w2 = pend
                n_groups = nkb // 2
                nc.tensor.matmul(
                    po_t[:],
                    V_sb[:, j0, :],
                    ex[:, 0:512],
                    start=(oi == 0),
                    stop=False,
                )
                nc.tensor.matmul(
                    po_t[:, 512 - w2 : 512],
                    V_sb[:, j0 + 1, :],
                    ex[:, _sl(512, w2)],
                    start=False,
                    stop=(oi == n_groups - 1),
                )
                if oi == n_groups - 1:
                    nc.vector.tensor_copy(out_sb[:, _sl(512 * qt, 512)], po_t[:])
                    nc.sync.dma_start(
                        out.ap()[:, _sl(512 * qt, 512)],
                        out_sb[:, _sl(512 * qt, 512)],
                    )

            slot_po = {}
            slot_oi = {}

            def pair(qt, j0, flush_lag=1):
                nkb = 2 * qt + 2
                if qt not in slot_po:
                    slot_po[qt] = ops.tile([E + 1, 512], f32, name="po")
                    slot_oi[qt] = 0
                oi = slot_oi[qt]
                slot_oi[qt] = oi + 1
                diag = j0 == nkb - 2
                w2 = 384 if diag else 512
                ps = sps.tile([P, 1024], f32, tag="ps")
                nc.tensor.matmul(
                    ps[:, 0:512],
                    kT_sb[0:E, _sl(P * j0, P)],
                    qT_sb[0:E, _sl(512 * qt, 512)],
                    start=True,
                    stop=True,
                    tile_position=(0, 0),
                )
                nc.tensor.matmul(
                    ps[:, _sl(512, w2)],
                    kT_sb[E:P, _sl(P * (j0 + 1), P)],
                    qT_sb[E:P, _sl(512 * qt + (512 - w2), w2)],
                    start=True,
                    stop=True,
                    tile_position=(64, 0),
                )
                ex = exppool.tile([P, 1024], fp16)
                nc.scalar.activation(
                    ex[:, : 512 + w2], ps[:, : 512 + w2], Exp, scale=0.125
                )
                if diag:
                    # 0/1 mask multiply on the (otherwise idle) gpsimd engine
                    nc.gpsimd.tensor_tensor(
                        out=ex[:, 0:896],
                        in0=ex[:, 0:896],
                        in1=maskD_sb[:],
                        op=Alu.mult,
                    )
                pendings.append((qt, oi, j0, nkb, ex, slot_po[qt], w2))
                if flush_lag is not None and len(pendings) > flush_lag:
                    issue_attnv(pendings.pop(0))

            def flush(n=None):
                k = len(pendings) if n is None else min(n, len(pendings))
                for _ in range(k):
                    issue_attnv(pendings.pop(0))

            # --- schedule
            # seg 0: stripe-0 projections (half-granular), slots 0,1
            kv_block_q0(0, xs0a_t)
            q_half_q0(0, xs0a_t)
            pair(0, 0)  # slot0 diag (needs dup0a only)
            kv_block_q0(1, xs0b_t)
            q_half_q0(1, xs0b_t)
            pair(1, 2)  # slot1 diag
            pair(1, 0)
            # segs 1..3: Q-half first, pairs interleaved into the proj
            q_half(1, 0)
            pair(2, 0)
            kv_stripe(1)
            pair(2, 2)
            q_half(1, 1)
            pair(2, 4)  # diag (dup1 ready)
            for j0 in (6, 0, 2, 4):  # slot3, diag first
                pair(3, j0)
            q_half(2, 0)
            pair(4, 0)
            kv_stripe(2)
            pair(4, 2)
            q_half(2, 1)
            pair(4, 8)  # diag
            pair(4, 4)
            pair(4, 6)
            for j0 in (10, 0, 2, 4, 6, 8):  # slot5, diag first
                pair(5, j0)
            q_half(3, 0)
            pair(6, 0)
            kv_stripe(3)
            pair(6, 2)
            q_half(3, 1)
            pair(6, 12)  # diag
            for j0 in (4, 6, 8, 10):
                pair(6, j0)
            for j0 in (14, 0, 2, 4, 6, 8, 10, 12):  # slot7, diag first
                pair(7, j0, flush_lag=2)
            flush()

    nc.compile()
    return nc


def _build_maskD(h):
    """Diagonal-pair causal mask [P, 896] fp16 (1=keep).

    cols 0..511  : q-col c vs k-tile j0       -> iql[c]     >= 256h + p
    cols 512..895: q-col 128+i vs k-tile j0+1 -> iql[128+i] >= 256h + 128 + p
    where iql[c] = c (h=0) or c^256 (h=1, parity block swap).
    """
    iql = np.arange(512, dtype=np.int64)
    if h == 1:
        iql = iql ^ 256
    p = np.arange(P, dtype=np.int64)
    m = np.zeros((P, 896), dtype=np.float16)
    m[:, 0:512] = iql[None, :] >= (HB * h + p)[:, None]
    m[:, 512:896] = iql[None, 128:512] >= (HB * h + P + p)[:, None]
    return m


def _ensure_axon_hooks_stub():
    """bass_utils imports antenv.axon_hooks when BASS_TRACE is set; that
    module is absent in this image, so provide a no-op registry."""
    try:
        import antenv.axon_hooks  # noqa: F401
    except ImportError:
        m = types.ModuleType("antenv.axon_hooks")
        m._h = [None]
        m.set_axon_ntff_profile_hook = lambda h: m._h.__setitem__(0, h)
        m.get_axon_ntff_profile_hook = lambda: m._h[0]
        sys.modules["antenv.axon_hooks"] = m


def kernel(x, Wq, Wk, Wv):
    _ensure_axon_hooks_stub()
    from concourse.bass_utils import run_bass_kernel_spmd

    if "nc" not in _cache:
        _cache["nc"] = _build_program()
    nc = _cache["nc"]

    x = np.asarray(x, dtype=np.float32)
    Wq = np.asarray(Wq, dtype=np.float32)
    Wk = np.asarray(Wk, dtype=np.float32)
    Wv = np.asarray(Wv, dtype=np.float32)

    wkv = np.ascontiguousarray(
        np.concatenate([Wk, Wv], axis=1)
        .reshape(DC, P, 2 * E)
        .transpose(1, 0, 2)
        .astype(np.float16)
    )
    wq2 = np.ascontiguousarray(
        np.concatenate([Wq, Wq], axis=1)
        .reshape(DC, P, P)
        .transpose(1, 0, 2)
        .astype(np.float16)
    )
    ones = np.ones((P, 16), dtype=np.float16)
    ident_np = np.zeros((P, 64), dtype=np.float16)
    for p_i in range(P):
        ident_np[p_i, p_i % 64] = 1.0
    maskDs = [_build_maskD(0), _build_maskD(1)]

    xT_all = x.transpose(0, 2, 1).astype(np.float16)  # [B, D, T]
    in_maps = []
    for c in range(NCORES):
        b, h = c // 2, c % 2
        xT = xT_all[b]
        if h == 1:  # swap 256-pairs so own-parity block is at even positions
            xT = xT.reshape(D, 8, 2, HB)[:, :, ::-1, :].reshape(D, T)
        # [dc, p, quarter, o] -> per-stripe [P, quarters, DC, HB] contiguous
        xq = xT.reshape(DC, P, 16, HB)
        im = {
            "xs0a": np.ascontiguousarray(xq[:, :, 0:2, :].transpose(1, 2, 0, 3)),
            "xs0b": np.ascontiguousarray(xq[:, :, 2:4, :].transpose(1, 2, 0, 3)),
            "wkv": wkv,
            "wq2": wq2,
            "maskD": maskDs[h],
            "ones": ones,
            "ident": ident_np,
        }
        for s in range(1, 4):
            im[f"xs{s}"] = np.ascontiguousarray(
                xq[:, :, 4 * s : 4 * s + 4, :].transpose(1, 2, 0, 3)
            )
        in_maps.append(im)

    res = run_bass_kernel_spmd(nc, in_maps, list(range(NCORES)))
    _cache["last_res"] = res

    outp = np.empty((B, T, E), dtype=np.float32)
    for b in range(B):
        U = np.zeros((E + 1, T), dtype=np.float64)
        for h in range(2):
            u = res.results[2 * b + h]["out"].astype(np.float64)
            if h == 1:
                u = u.reshape(E + 1, 8, 2, HB)[:, :, ::-1, :].reshape(E + 1, T)
            U += u
        outp[b] = (U[:E] / U[E : E + 1]).T.astype(np.float32)
    return outp
